# revision 21
# baseline (speedup 1.0000x reference)
"""GAT (graph attention) layer on 8 TRN2 NeuronCores — Bass/Tile kernel.

v3: host supplies hT (= h.T) and hlT (local shard of hT) so no PE
transposes of h are needed; Wh is computed locally per core (replicated)
directly from hT.  The Wh column-tile loop (phase 1) and the per-j-tile
logits/softmax/aggregation loop (phase 2) are software-pipelined in the
emission order (Wh(s) | logits+chain(s-2) | aggregation(s-4)) so every
engine queue streams without cross-phase stalls.

Math (per core, rows i in its shard):
  Wh = h @ W                                  [2048, 8, 64]
  e_i[i,h] = <Wh[i,h,:], a_i[h,:]> ;  e_j[j,h] likewise
  P^T[j,(h,i)] = adj[i,j] * exp(leaky_relu(e_i[h,i] + e_j[j,h]))
  out[i,(h,d)] = elu( (P^T.T @ [Wh_h | 1])[:, :64] / denom )

The logits tile y[j,(h,i)] = e_i + e_j is RANK-9: one K=9 matmul per
j-tile materializes all 8 heads at once.  A ones-column appended to Wh
gives the softmax denominator for free.

The leaky-relu/exp/mask chain is balanced across ACT and DVE: most
halves use {ACT: t=0.2*z, DVE: max(t, z), ACT: exp} (route B); a few use
an all-DVE f16 variant (route A) to even out engine load.
"""

import dataclasses
import sys

import numpy as np

sys.path.insert(0, "/opt/trn_rl_repo")

N = 2048
F_IN = 768
F_OUT = 64
H = 8
ALPHA = 0.2
NCORES = 8
NL = N // NCORES          # 256 local rows per core
KT = F_IN // 128          # 6 k-tiles
NT = N // 128              # 16 n/j tiles
FH = F_OUT * H            # 512
FE = FH + H               # 520: [W | wa_j]

MM_DT = "float32r"
ROUTE_A = frozenset({(3, 1), (7, 1), (11, 1), (15, 1)})
LAG_Y = 2                 # y/chain trails Wh by this many tiles
LAG_G = 4                 # aggregation trails Wh by this many tiles

_CACHE = {}


def _build():
    import concourse.bacc as bacc
    import concourse.mybir as mybir
    from concourse.tile import TileContext

    f32 = mybir.dt.float32
    bf16 = mybir.dt.bfloat16
    f16 = mybir.dt.float16
    mmdt = getattr(mybir.dt, MM_DT)
    AF = mybir.ActivationFunctionType
    OP = mybir.AluOpType

    nc = bacc.Bacc("TRN2", target_bir_lowering=False, debug=False,
                   num_devices=NCORES)

    hT_d = nc.declare_dram_parameter("hT", [F_IN, N], f32, isOutput=False)
    hlT_d = nc.declare_dram_parameter("hlT", [F_IN, NL], f32, isOutput=False)
    adjT_d = nc.declare_dram_parameter("adjT", [N, NL], f32, isOutput=False)
    W_d = nc.declare_dram_parameter("W", [F_IN, FH], f32, isOutput=False)
    ai_d = nc.declare_dram_parameter("a_i", [1, FH], f32, isOutput=False)
    aj_d = nc.declare_dram_parameter("a_j", [1, FH], f32, isOutput=False)
    out_d = nc.declare_dram_parameter("out", [NL, FH], f32, isOutput=True)

    HW2 = H // 2 * NL      # 1024 cols: 4 heads x 256 i per half

    with TileContext(nc) as tc:
        with tc.tile_pool(name="persist", bufs=1) as pp:
            ident = pp.tile([128, 128], f32)
            W_sb = pp.tile([128, KT, FH], f32)
            W_r = pp.tile([128, KT, FE], mmdt)
            hT_q = [pp.tile([128, KT, 512], mmdt, name=f"hT{q}")
                    for q in range(4)]
            hlT_sb = pp.tile([128, KT, NL], mmdt)
            Wh_t = [pp.tile([128, H, F_OUT + 1], bf16, name=f"Wh{t}")
                    for t in range(NT)]
            ejT_t = [pp.tile([H + 1, 128], mmdt, name=f"ejT{t}")
                     for t in range(NT)]
            adjT_b = pp.tile([128, NT, NL], bf16)
            rhs_sb = pp.tile([H + 1, H * NL], mmdt)
            eiT_sb = pp.tile([H, NL], mmdt)
            ai_bc = pp.tile([128, FH], f32)
            aj_bc = pp.tile([128, FH], f32)
            ones_row = pp.tile([1, 128], f32)
            hp_sb = pp.tile([128, 2, FH], f32)
            out_sb = pp.tile([128, 2, FH], f32)
            r_sb = pp.tile([128, NT], f32)
            dn_sb = pp.tile([128, NT], f32)
            zs_row = pp.tile([1, 512], f32)

            # ---------- input DMAs (order = Sync-queue order) ----------
            for k in range(KT):
                nc.sync.dma_start(out=W_sb[:, k, :],
                                  in_=W_d[k * 128:(k + 1) * 128, :])
            nc.sync.dma_start(
                out=hlT_sb[:],
                in_=hlT_d[:].rearrange("(k p) n -> p k n",
                                       p=128).bitcast(mmdt))

            with tc.tile_pool(name="ph1", bufs=3) as sp, \
                 tc.tile_pool(name="adjs", bufs=2) as ap_, \
                 tc.tile_pool(name="ring", bufs=2, space="PSUM") as ps, \
                 tc.tile_pool(name="tp", bufs=1, space="PSUM") as tpp, \
                 tc.tile_pool(name="psagg", bufs=1, space="PSUM") as pap, \
                 tc.tile_pool(name="ebuf", bufs=2) as eb:

                ps_agg = pap.tile([128, NT, F_OUT + 1], f32)

                a_t = sp.tile([1, FH], f32, tag="a")
                nc.sync.dma_start(out=a_t[:], in_=ai_d[:])
                a2_t = sp.tile([1, FH], f32, tag="a")
                nc.sync.dma_start(out=a2_t[:], in_=aj_d[:])
                # adjT group 0 early (needed by the first chain halves)
                adjf = [ap_.tile([128, 4, NL], f32, tag="adjf",
                                 name=f"adjf{g}")
                        for g in range(4)]
                nc.sync.dma_start(
                    out=adjf[0][:],
                    in_=adjT_d[0:512, :].rearrange("(t p) i -> p t i", p=128))
                # bulk hT stream (4 chunks of 512 j-columns)
                for q in range(4):
                    nc.sync.dma_start(
                        out=hT_q[q][:],
                        in_=hT_d[:, q * 512:(q + 1) * 512].rearrange(
                            "(k p) n -> p k n", p=128).bitcast(mmdt))
                for g in range(1, 4):
                    nc.sync.dma_start(
                        out=adjf[g][:],
                        in_=adjT_d[g * 512:(g + 1) * 512, :].rearrange(
                            "(t p) i -> p t i", p=128))

                # gpsimd: iotas first (they gate DVE constants)
                io_t = sp.tile([128, 128], mybir.dt.int32, tag="iota", bufs=1)
                nc.gpsimd.iota(io_t[:], pattern=[[-1, 128]], base=0,
                               channel_multiplier=1)
                io_r = sp.tile([H + 1, H, NL], mybir.dt.int32, tag="iotar", bufs=1)
                nc.gpsimd.iota(io_r[:], pattern=[[-1, H], [0, NL]], base=0,
                               channel_multiplier=1)

                nc.vector.tensor_scalar(ident[:], io_t[:], 0, None,
                                        OP.is_equal)
                nc.vector.memset(ones_row[:], 1.0)
                nc.vector.memset(zs_row[:], 0.0)
                nc.vector.tensor_scalar(
                    rhs_sb[:].rearrange("p (h i) -> p h i", h=H),
                    io_r[:], 0, None, OP.is_equal)

                # HAM warm-up: ~12 back-to-back wide matmuls gated on the
                # W DMA, so the PE sees one sustained-busy SHORT window
                # right before the dense Wh stream begins.  They scribble
                # into the agg banks, which the real zeroing matmuls
                # below then clear.
                agg_flat = ps_agg[:].rearrange("p g d -> p (g d)")
                for r in range(12):
                    nc.tensor.matmul(agg_flat[:, 0:512],
                                     ones_row[0:1, 0:128],
                                     W_sb[0:1, 0, 0:512],
                                     start=True, stop=True,
                                     skip_group_check=True)
                # zero agg banks via dummy matmuls
                tot = NT * (F_OUT + 1)
                off = 0
                while off < tot:
                    w = min(512, tot - off)
                    nc.tensor.matmul(agg_flat[:, off:off + w],
                                     zs_row[0:1, 0:128],
                                     zs_row[0:1, 0:w],
                                     start=True, stop=False,
                                     skip_group_check=True)
                    off += w

                # broadcast a_i/a_j to 128 partitions (K=1 matmul)
                for src, dst in ((a_t, ai_bc), (a2_t, aj_bc)):
                    ps_b = ps.tile([128, HW2], f32, tag="y")
                    nc.tensor.matmul(ps_b[:, 0:FH], ones_row[:], src[:],
                                     start=True, stop=True)
                    nc.scalar.copy(out=dst[:], in_=ps_b[:, 0:FH])

                # W -> f32r (pure-W columns); wa_j folded into cols 512:520
                for k in range(KT):
                    nc.vector.tensor_copy(W_r[:, k, 0:FH], W_sb[:, k, :])
                for k in range(KT):
                    t_t = sp.tile([128, FH], f32, tag="wtmp", bufs=2)
                    nc.vector.tensor_tensor(t_t[:], W_sb[:, k, :], aj_bc[:],
                                            OP.mult)
                    with nc.allow_low_precision(
                            reason="wa_j lands in the f32r matmul operand; "
                                   "f32r rounding applies regardless"):
                        nc.vector.tensor_reduce(
                            W_r[:, k, FH:FE],
                            t_t[:].rearrange("p (h d) -> p h d", h=H),
                            mybir.AxisListType.X, OP.add)

                # e_i from a local Wh matmul (no wa_i fold needed)
                for lt in range(NL // 128):
                    ps_wl = ps.tile([128, HW2], f32, tag="y")
                    for k in range(KT):
                        nc.tensor.matmul(
                            ps_wl[:, 0:FH],
                            hlT_sb[:, k, lt * 128:(lt + 1) * 128],
                            W_r[:, k, 0:FH],
                            start=(k == 0), stop=(k == KT - 1))
                    t_t = sp.tile([128, FH], f32, tag="wtmp", bufs=2)
                    nc.vector.tensor_tensor(t_t[:], ps_wl[:, 0:FH],
                                            ai_bc[:], OP.mult)
                    ei_t = sp.tile([128, H], f32, tag="ei")
                    nc.vector.tensor_reduce(
                        ei_t[:], t_t[:].rearrange("p (h d) -> p h d", h=H),
                        mybir.AxisListType.X, OP.add)
                    ps_e = tpp.tile([H, 128], f32, tag="tp")
                    nc.tensor.transpose(ps_e[:], ei_t[:], ident[:])
                    nc.vector.tensor_copy(
                        eiT_sb[:, lt * 128:(lt + 1) * 128], ps_e[:])
                for hh in range(H):
                    nc.sync.dma_start(
                        out=rhs_sb[H:H + 1, hh * NL:(hh + 1) * NL],
                        in_=eiT_sb[hh:hh + 1, :])

                # adjT f32 -> bf16 casts (split between DVE and ACT)
                for g in range(4):
                    dst = adjT_b[:, g * 4:(g + 1) * 4, :]
                    if g % 2 == 0:
                        nc.vector.tensor_copy(dst, adjf[g][:])
                    else:
                        nc.scalar.copy(out=dst, in_=adjf[g][:])

                # ---------- software-pipelined main loops ----------
                # Half-tile granularity: each half-step emits ~2.3us of
                # work per engine so the PE never starves (HAM stays warm).
                wh_ps = {}

                def emit_wh_half(t, half):
                    q, qo = divmod(t, 4)
                    if half == 0:
                        wh_ps[t] = ps.tile([128, HW2], f32, tag="y", name=f"psw{t}")
                    ps_w = wh_ps[t]
                    for k in range(3 * half, 3 * half + 3):
                        lhs = hT_q[q][:, k, qo * 128:(qo + 1) * 128]
                        nc.tensor.matmul(ps_w[:, 0:512], lhs,
                                         W_r[:, k, 0:512],
                                         start=(k == 0), stop=(k == KT - 1))
                        nc.tensor.matmul(ps_w[:, 512:FE], lhs,
                                         W_r[:, k, 512:FE],
                                         start=(k == 0), stop=(k == KT - 1))
                    if half == 1:
                        ps_w = wh_ps.pop(t)
                        nc.scalar.copy(
                            out=Wh_t[t][:, :, 0:F_OUT],
                            in_=ps_w[:, 0:FH].rearrange("p (h d) -> p h d",
                                                        h=H))
                        nc.gpsimd.memset(Wh_t[t][:, :, F_OUT:F_OUT + 1], 1.0)
                        ej_t = sp.tile([128, H], f32, tag="ej")
                        nc.vector.tensor_copy(ej_t[:], ps_w[:, FH:FE])
                        ps_e = tpp.tile([H, 128], f32, tag="tp")
                        nc.tensor.transpose(ps_e[:], ej_t[:], ident[:])
                        nc.gpsimd.memset(ejT_t[t][:].bitcast(f32), 1.0)
                        nc.vector.tensor_copy(ejT_t[t][0:H, :], ps_e[:])

                E_tiles = {}

                def emit_chain_half(jt, hf):
                    f0 = hf * HW2
                    ps_y = ps.tile([128, HW2], f32, tag="y")
                    for q in range(2):
                        nc.tensor.matmul(
                            ps_y[:, q * 512:(q + 1) * 512],
                            ejT_t[jt][:],
                            rhs_sb[:, f0 + q * 512:f0 + (q + 1) * 512],
                            start=True, stop=True)
                    E_t = eb.tile([128, HW2], bf16, tag="E", bufs=6)
                    if (jt, hf) in ROUTE_A:
                        t16a = eb.tile([128, HW2], f16, tag="ta")
                        nc.vector.tensor_scalar(t16a[:], ps_y[:], ALPHA,
                                                None, OP.mult)
                        t16b = eb.tile([128, HW2], f16, tag="tb")
                        nc.vector.tensor_copy(t16b[:], ps_y[:])
                        L16 = eb.tile([128, HW2], f16, tag="L16")
                        nc.vector.tensor_tensor(L16[:], t16a[:],
                                                t16b[:], OP.max)
                        nc.scalar.activation(E_t[:], L16[:], AF.Exp)
                    else:
                        t_t = eb.tile([128, HW2], f16, tag="Ls")
                        with nc.allow_low_precision(
                                reason="0.2*z scale copy; f16 ulp only "
                                       "affects the negative lrelu arm"):
                            nc.scalar.mul(out=t_t[:], in_=ps_y[:],
                                          mul=ALPHA)
                        L_t = eb.tile([128, HW2], f32, tag="L")
                        nc.vector.tensor_tensor(L_t[:], t_t[:], ps_y[:],
                                                OP.max)
                        nc.scalar.activation(E_t[:], L_t[:], AF.Exp)
                    base = adjT_b[:, jt, :]
                    rep = dataclasses.replace(
                        base, ap=[list(base.ap[0]), [0, H // 2],
                                  list(base.ap[1])])
                    nc.vector.tensor_tensor(
                        E_t[:].rearrange("p (h i) -> p h i", h=H // 2),
                        E_t[:].rearrange("p (h i) -> p h i", h=H // 2),
                        rep, OP.mult)
                    E_tiles[(jt, hf)] = E_t

                def emit_agg_half(jt, hf):
                    E_t = E_tiles.pop((jt, hf))
                    for hh in range(H // 2):
                        for ih in range(2):
                            hg = hf * (H // 2) + hh
                            g = hg * 2 + ih
                            nc.tensor.matmul(
                                ps_agg[:, g, :],
                                E_t[:, hh * NL + ih * 128:
                                    hh * NL + ih * 128 + 128],
                                Wh_t[jt][:, hg, :],
                                start=False, stop=(jt == NT - 1),
                                skip_group_check=True)

                NH = 2 * NT
                for hs in range(NH + 2 * LAG_G):
                    s, half = divmod(hs, 2)
                    if s < NT:
                        emit_wh_half(s, half)
                    jy, hf = divmod(hs - 2 * LAG_Y, 2)
                    if 0 <= jy < NT:
                        emit_chain_half(jy, hf)
                    jg, hg_ = divmod(hs - 2 * LAG_G, 2)
                    if 0 <= jg < NT:
                        emit_agg_half(jg, hg_)

                # ---------- finalize: normalize + ELU + store ----------
                nc.vector.tensor_copy(dn_sb[:], ps_agg[:, :, F_OUT])
                nc.vector.reciprocal(r_sb[:], dn_sb[:])
                for hh in range(H):
                    for ih in range(2):
                        g = hh * 2 + ih
                        nc.vector.tensor_scalar(
                            hp_sb[:, ih, hh * F_OUT:(hh + 1) * F_OUT],
                            ps_agg[:, g, 0:F_OUT],
                            r_sb[:, g:g + 1], None, OP.mult)
                hp_flat = hp_sb[:].rearrange("p a c -> p (a c)")
                mn_t = eb.tile([128, 2 * FH], f32, tag="L")
                nc.vector.tensor_scalar(mn_t[:], hp_flat, 0.0, None, OP.min)
                em_t = eb.tile([128, 2 * FH], f32, tag="L")
                nc.scalar.activation(em_t[:], mn_t[:], AF.Exp)
                nc.vector.scalar_tensor_tensor(
                    out_sb[:].rearrange("p a c -> p (a c)"), em_t[:], -1.0,
                    hp_flat, OP.add, OP.max)
                for ih in range(2):
                    nc.sync.dma_start(out=out_d[ih * 128:(ih + 1) * 128, :],
                                      in_=out_sb[:, ih, :])

    nc.compile()
    return nc


def kernel(h, adj, W, a):
    from concourse.bass_utils import run_bass_kernel_spmd

    if "nc" not in _CACHE:
        _CACHE["nc"] = _build()
    nc = _CACHE["nc"]

    h = np.ascontiguousarray(h, dtype=np.float32)
    adj = np.ascontiguousarray(adj, dtype=np.float32)
    W = np.ascontiguousarray(W, dtype=np.float32)
    a = np.ascontiguousarray(a, dtype=np.float32)
    a_i = np.ascontiguousarray(a[0, :, :F_OUT].reshape(1, FH))
    a_j = np.ascontiguousarray(a[0, :, F_OUT:].reshape(1, FH))
    hT = np.ascontiguousarray(h.T)

    in_maps = []
    for c in range(NCORES):
        sl = slice(c * NL, (c + 1) * NL)
        in_maps.append({
            "hT": hT,
            "hlT": np.ascontiguousarray(hT[:, sl]),
            "adjT": np.ascontiguousarray(adj[sl].T),
            "W": W,
            "a_i": a_i,
            "a_j": a_j,
        })
    res = run_bass_kernel_spmd(nc, in_maps, list(range(NCORES)),
                               trace=bool(_CACHE.get("trace")))
    _CACHE["last"] = res
    return np.concatenate([res.results[c]["out"] for c in range(NCORES)],
                          axis=0)
